# revision 1
# baseline (speedup 1.0000x reference)
"""Segment-mean (MeanAggregator) Trainium2 kernel.

Problem: atom_hiddens [2_000_000, 128] f32, segment_ids = repeat(arange(100_000), 20)
(uniform 20 atoms per molecule), output = per-molecule mean [100_000, 128] f32.

Strategy (8 NeuronCores, data-parallel over molecules):
  - Each core handles 12_500 molecules = 250_000 contiguous atom rows (128 MB).
  - Molecule-per-partition layout: partition p of a group holds molecule
    g*128+p as 20 contiguous rows (r, h) = 2560 f32 = 10 KB.  One DMA per
    super-tile (4 groups, 5.24 MB) with fully-contiguous 10 KB per-partition
    runs -> near-peak HBM bandwidth (512 B-chunk layouts measured ~290 GB/s;
    contiguous runs ~355 GB/s).
  - The 20-chunk reduction sum_r tile[p, g, r, :] is split between two engines
    so neither is the bottleneck:
      * PE groups: 20 accumulating fp32 matmuls with an IDENTITY stationary
        operand (partition-preserving accumulate into PSUM).  fp32 matmul is
        4 cycles/row (exact); FD=256 (2 groups) per instruction.
      * DVE groups: tensor_reduce over a permuted AP [p, h, r] (axis=X).
    Both are exact fp32 adds.  ScalarE evicts/scales by 1/20; output DMAs go
    out on the second HWDGE ring (nc.scalar) to keep the SP ring input-only.
  - Tail per core: 212 mols = one 128-mol group + one 84-mol group, both via
    identity matmuls FD=128.
"""

import numpy as np

N_CORES = 8
TOTAL_ATOMS = 2_000_000
HIDDEN = 128
N_MOLS = 100_000
K = 20  # atoms per molecule
MOLS_PER_CORE = N_MOLS // N_CORES  # 12_500
ATOMS_PER_CORE = TOTAL_ATOMS // N_CORES  # 250_000

G = 4  # groups per super-tile
PE_G = 1  # groups 0..PE_G-1 reduced on PE; the rest on DVE (one fused reduce)
MOLS_PER_GROUP = 128
ATOMS_PER_GROUP = MOLS_PER_GROUP * K  # 2560
MOLS_PER_ST = G * MOLS_PER_GROUP  # 512
ATOMS_PER_ST = G * ATOMS_PER_GROUP  # 10240
N_ST = MOLS_PER_CORE // MOLS_PER_ST  # 24 full super-tiles
TAIL_MOLS = MOLS_PER_CORE - N_ST * MOLS_PER_ST  # 212
TAIL_A_MOLS = 128
TAIL_B_MOLS = TAIL_MOLS - TAIL_A_MOLS  # 84

_CACHE = {}


def _build_program():
    import concourse.bacc as bacc
    import concourse.tile as tile
    from concourse import mybir

    nc = bacc.Bacc("TRN2", target_bir_lowering=False, debug=False)

    f32 = mybir.dt.float32

    x = nc.dram_tensor("x", [ATOMS_PER_CORE, HIDDEN], f32, kind="ExternalInput")
    ident = nc.dram_tensor("ident", [128, 128], f32, kind="ExternalInput")
    y = nc.dram_tensor("y", [MOLS_PER_CORE, HIDDEN], f32, kind="ExternalOutput")

    inv_k = 1.0 / K
    copy = mybir.ActivationFunctionType.Copy
    AX = mybir.AxisListType.X

    with tile.TileContext(nc) as tc:
        with (
            tc.tile_pool(name="constp", bufs=1) as constp,
            tc.tile_pool(name="inp_pe", bufs=2) as inp_pe,
            tc.tile_pool(name="inp_dve", bufs=2) as inp_dve,
            tc.tile_pool(name="outp", bufs=2) as outp,
            tc.tile_pool(name="psump", bufs=2, space="PSUM") as psump,
        ):
            ident_sb = constp.tile([128, 128], f32)
            nc.sync.dma_start(out=ident_sb, in_=ident[:, :])

            # ---- main super-tiles ----
            # Alternate whole super-tiles between the two reduction engines so
            # each engine's work stream is dense (PE stays HAM-warm) and DMA is
            # the only pacer: PE stream ~20 us/ST, DVE stream ~18 us/ST, DMA
            # ~12.5 us/ST with 12 STs each.
            for s in range(N_ST):
                a0 = s * ATOMS_PER_ST
                on_pe = s % 2 == 0
                pool = inp_pe if on_pe else inp_dve
                in_t = pool.tile([128, G, K, HIDDEN], f32)
                nc.sync.dma_start(
                    out=in_t,
                    in_=x[a0 : a0 + ATOMS_PER_ST, :].rearrange(
                        "(g p r) h -> p g r h", g=G, p=128, r=K
                    ),
                )

                o_t = outp.tile([128, G, HIDDEN], f32, tag="out")

                if on_pe:
                    ps = psump.tile([128, G * HIDDEN], f32)
                    for r in range(K):
                        nc.tensor.matmul(
                            ps,
                            lhsT=ident_sb,
                            rhs=in_t[:, :, r, :],
                            start=(r == 0),
                            stop=(r == K - 1),
                        )
                    nc.scalar.activation(o_t, ps, copy, scale=inv_k)
                else:
                    for g in range(G):
                        nc.vector.reduce_sum(
                            out=o_t[:, g, :],
                            in_=in_t[:, g, :, :].rearrange("p r h -> p h r"),
                            axis=AX,
                        )
                    nc.vector.tensor_scalar_mul(o_t, o_t, inv_k)

                dst = y[s * MOLS_PER_ST : (s + 1) * MOLS_PER_ST, :].rearrange(
                    "(g p) h -> p g h", g=G, p=128
                )
                nc.scalar.dma_start(out=dst, in_=o_t)

            # ---- tail A: 128 mols ----
            a0 = N_ST * ATOMS_PER_ST
            m0 = N_ST * MOLS_PER_ST
            in_a = inp_pe.tile([128, K, HIDDEN], f32)
            nc.sync.dma_start(
                out=in_a,
                in_=x[a0 : a0 + ATOMS_PER_GROUP, :].rearrange(
                    "(p r) h -> p r h", p=128, r=K
                ),
            )
            ps_a = psump.tile([128, HIDDEN], f32)
            for r in range(K):
                nc.tensor.matmul(
                    ps_a,
                    lhsT=ident_sb,
                    rhs=in_a[:, r, :],
                    start=(r == 0),
                    stop=(r == K - 1),
                )
            o_a = outp.tile([128, HIDDEN], f32, tag="out")
            nc.scalar.activation(o_a, ps_a, copy, scale=inv_k)
            nc.scalar.dma_start(out=y[m0 : m0 + TAIL_A_MOLS, :], in_=o_a)

            # ---- tail B: 84 mols ----
            a0 += ATOMS_PER_GROUP
            m0 += TAIL_A_MOLS
            in_b = inp_pe.tile([128, K, HIDDEN], f32)
            nc.sync.dma_start(
                out=in_b[:TAIL_B_MOLS],
                in_=x[a0 : a0 + TAIL_B_MOLS * K, :].rearrange(
                    "(p r) h -> p r h", p=TAIL_B_MOLS, r=K
                ),
            )
            ps_b = psump.tile([128, HIDDEN], f32)
            for r in range(K):
                nc.tensor.matmul(
                    ps_b[:TAIL_B_MOLS, :],
                    lhsT=ident_sb[:TAIL_B_MOLS, :TAIL_B_MOLS],
                    rhs=in_b[:TAIL_B_MOLS, r, :],
                    start=(r == 0),
                    stop=(r == K - 1),
                )
            o_b = outp.tile([128, HIDDEN], f32, tag="out")
            nc.scalar.activation(
                o_b[:TAIL_B_MOLS, :], ps_b[:TAIL_B_MOLS, :], copy, scale=inv_k
            )
            nc.scalar.dma_start(
                out=y[m0 : m0 + TAIL_B_MOLS, :], in_=o_b[:TAIL_B_MOLS, :]
            )

    nc.finalize()
    return nc


def _get_program():
    if "nc" not in _CACHE:
        _CACHE["nc"] = _build_program()
    return _CACHE["nc"]


def _uniform_pattern(segment_ids: np.ndarray, n_mols: int) -> bool:
    if segment_ids.shape != (TOTAL_ATOMS,) or n_mols != N_MOLS:
        return False
    expect = np.repeat(np.arange(N_MOLS, dtype=segment_ids.dtype), K)
    return bool(np.array_equal(segment_ids, expect))


def _numpy_fallback(atom_hiddens, segment_ids, n_mols):
    """Correct-but-slow path for non-uniform segment layouts (sorted ids)."""
    ah = np.asarray(atom_hiddens, dtype=np.float32)
    sid = np.asarray(segment_ids).astype(np.int64)
    counts = np.bincount(sid, minlength=n_mols).astype(np.float32)
    boundaries = np.searchsorted(sid, np.arange(n_mols))
    sums = np.add.reduceat(ah, boundaries, axis=0)
    empty = counts == 0
    if empty.any():
        sums[empty] = 0.0
    return sums / np.maximum(counts, 1.0)[:, None]


def kernel(**inputs) -> np.ndarray:
    atom_hiddens = np.asarray(inputs["atom_hiddens"], dtype=np.float32)
    segment_ids = np.asarray(inputs["segment_ids"])
    n_mols = int(np.asarray(inputs["n_mols"]))

    if not _uniform_pattern(segment_ids, n_mols) or atom_hiddens.shape != (
        TOTAL_ATOMS,
        HIDDEN,
    ):
        return _numpy_fallback(atom_hiddens, segment_ids, n_mols)

    from concourse.bass_utils import run_bass_kernel_spmd

    nc = _get_program()
    ident = np.eye(128, dtype=np.float32)
    in_maps = [
        {
            "x": atom_hiddens[c * ATOMS_PER_CORE : (c + 1) * ATOMS_PER_CORE],
            "ident": ident,
        }
        for c in range(N_CORES)
    ]
    res = run_bass_kernel_spmd(nc, in_maps, core_ids=list(range(N_CORES)))
    return np.concatenate([r["y"] for r in res.results], axis=0)


if __name__ == "__main__":
    rng = np.random.default_rng(0)
    ah = rng.standard_normal((TOTAL_ATOMS, HIDDEN), dtype=np.float32)
    sid = np.repeat(np.arange(N_MOLS, dtype=np.int32), K)
    out = kernel(atom_hiddens=ah, segment_ids=sid, n_mols=N_MOLS)
    ref = ah.reshape(N_MOLS, K, HIDDEN).mean(axis=1)
    err = np.abs(out - ref).max() / max(np.abs(ref).max(), 1e-9)
    print("rel err:", err)



# revision 10
# speedup vs baseline: 1.1255x; 1.1255x over previous
"""Segment-mean (MeanAggregator) Trainium2 kernel.

Problem: atom_hiddens [2_000_000, 128] f32, segment_ids = repeat(arange(100_000), 20)
(uniform 20 atoms per molecule), output = per-molecule mean [100_000, 128] f32.

Strategy (8 NeuronCores, data-parallel over molecules):
  - Each core handles 12_500 molecules = 250_000 contiguous atom rows (128 MB).
  - Molecule-per-partition layout: partition p of a group holds molecule
    g*128+p as 20 contiguous rows (r, h) = 2560 f32 = 10 KB.  One DMA per
    super-tile (4 groups, 5.24 MB) with fully-contiguous 10 KB per-partition
    runs -> near-peak HBM bandwidth (512 B-chunk layouts measured ~290 GB/s;
    contiguous runs ~355 GB/s).
  - The 20-chunk reduction sum_r tile[p, g, r, :] is split between two engines
    so neither is the bottleneck:
      * PE groups: 20 accumulating fp32 matmuls with an IDENTITY stationary
        operand (partition-preserving accumulate into PSUM).  fp32 matmul is
        4 cycles/row (exact); FD=256 (2 groups) per instruction.
      * DVE groups: tensor_reduce over a permuted AP [p, h, r] (axis=X).
    Both are exact fp32 adds.  ScalarE evicts/scales by 1/20; output DMAs go
    out on the second HWDGE ring (nc.scalar) to keep the SP ring input-only.
  - Tail per core: 212 mols = one 128-mol group + one 84-mol group, both via
    identity matmuls FD=128.
"""

import numpy as np

N_CORES = 8
TOTAL_ATOMS = 2_000_000
HIDDEN = 128
N_MOLS = 100_000
K = 20  # atoms per molecule
MOLS_PER_CORE = N_MOLS // N_CORES  # 12_500
ATOMS_PER_CORE = TOTAL_ATOMS // N_CORES  # 250_000

G = 2  # groups per super-tile
MOLS_PER_GROUP = 128
ATOMS_PER_GROUP = MOLS_PER_GROUP * K  # 2560
MOLS_PER_ST = G * MOLS_PER_GROUP  # 256
ATOMS_PER_ST = G * ATOMS_PER_GROUP  # 5120
N_ST = MOLS_PER_CORE // MOLS_PER_ST  # 48 full super-tiles
TAIL_MOLS = MOLS_PER_CORE - N_ST * MOLS_PER_ST  # 212
TAIL_A_MOLS = 128
TAIL_B_MOLS = TAIL_MOLS - TAIL_A_MOLS  # 84

_CACHE = {}


def _build_program():
    import concourse.bacc as bacc
    import concourse.tile as tile
    from concourse import mybir

    nc = bacc.Bacc("TRN2", target_bir_lowering=False, debug=False)

    f32 = mybir.dt.float32

    x = nc.dram_tensor("x", [ATOMS_PER_CORE, HIDDEN], f32, kind="ExternalInput")
    ident = nc.dram_tensor("ident", [128, 128], f32, kind="ExternalInput")
    y = nc.dram_tensor("y", [MOLS_PER_CORE, HIDDEN], f32, kind="ExternalOutput")

    inv_k = 1.0 / K
    copy = mybir.ActivationFunctionType.Copy
    AX = mybir.AxisListType.X

    with tile.TileContext(nc) as tc:
        with (
            tc.tile_pool(name="constp", bufs=1) as constp,
            tc.tile_pool(name="inp_pe", bufs=4) as inp_pe,
            tc.tile_pool(name="inp_dve", bufs=4) as inp_dve,
            tc.tile_pool(name="outp", bufs=4) as outp,
            tc.tile_pool(name="tailp", bufs=1) as tailp,
            tc.tile_pool(name="psump", bufs=4, space="PSUM") as psump,
            tc.tile_pool(name="tailps", bufs=1, space="PSUM") as tailps,
        ):
            ident_sb = constp.tile([128, 128], f32)
            nc.scalar.dma_start(out=ident_sb, in_=ident[:, :])

            # ---- main super-tiles ----
            # Alternate whole super-tiles between the two reduction engines.
            # Small STs (2.62 MB) + 4-deep input pools keep the SDMA queue fed:
            # with bufs=2 the next round's DMA issue waited on compute freeing a
            # buffer (traced 13-15 us SDMA idle per round).  Per ST: DMA ~6.4 us,
            # PE ~9.7 us, DVE ~9.2 us; each engine gets an ST every ~12.8 us, so
            # DMA is the sole pacer.
            for s in range(N_ST):
                a0 = s * ATOMS_PER_ST
                on_pe = s % 2 == 0
                pool = inp_pe if on_pe else inp_dve
                in_t = pool.tile([128, G, K, HIDDEN], f32)
                nc.sync.dma_start(
                    out=in_t,
                    in_=x[a0 : a0 + ATOMS_PER_ST, :].rearrange(
                        "(g p r) h -> p g r h", g=G, p=128, r=K
                    ),
                )

                o_t = outp.tile([128, G, HIDDEN], f32, tag="out")

                if on_pe:
                    ps = psump.tile([128, G * HIDDEN], f32)
                    for r in range(K):
                        nc.tensor.matmul(
                            ps,
                            lhsT=ident_sb,
                            rhs=in_t[:, :, r, :],
                            start=(r == 0),
                            stop=(r == K - 1),
                        )
                    nc.scalar.activation(o_t, ps, copy, scale=inv_k)
                else:
                    for g in range(G):
                        nc.vector.reduce_sum(
                            out=o_t[:, g, :],
                            in_=in_t[:, g, :, :].rearrange("p r h -> p h r"),
                            axis=AX,
                        )
                    nc.vector.tensor_scalar_mul(o_t, o_t, inv_k)

                dst = y[s * MOLS_PER_ST : (s + 1) * MOLS_PER_ST, :].rearrange(
                    "(g p) h -> p g h", g=G, p=128
                )
                nc.scalar.dma_start(out=dst, in_=o_t)

            # ---- tail A: 128 mols ----
            a0 = N_ST * ATOMS_PER_ST
            m0 = N_ST * MOLS_PER_ST
            in_a = tailp.tile([128, K, HIDDEN], f32)
            nc.sync.dma_start(
                out=in_a,
                in_=x[a0 : a0 + ATOMS_PER_GROUP, :].rearrange(
                    "(p r) h -> p r h", p=128, r=K
                ),
            )
            ps_a = tailps.tile([128, HIDDEN], f32)
            for r in range(K):
                nc.tensor.matmul(
                    ps_a,
                    lhsT=ident_sb,
                    rhs=in_a[:, r, :],
                    start=(r == 0),
                    stop=(r == K - 1),
                )
            o_a = outp.tile([128, HIDDEN], f32, tag="out")
            nc.scalar.activation(o_a, ps_a, copy, scale=inv_k)
            nc.scalar.dma_start(out=y[m0 : m0 + TAIL_A_MOLS, :], in_=o_a)

            # ---- tail B: 84 mols ----
            a0 += ATOMS_PER_GROUP
            m0 += TAIL_A_MOLS
            in_b = tailp.tile([128, K, HIDDEN], f32)
            nc.sync.dma_start(
                out=in_b[:TAIL_B_MOLS],
                in_=x[a0 : a0 + TAIL_B_MOLS * K, :].rearrange(
                    "(p r) h -> p r h", p=TAIL_B_MOLS, r=K
                ),
            )
            ps_b = tailps.tile([128, HIDDEN], f32)
            for r in range(K):
                nc.tensor.matmul(
                    ps_b[:TAIL_B_MOLS, :],
                    lhsT=ident_sb[:TAIL_B_MOLS, :TAIL_B_MOLS],
                    rhs=in_b[:TAIL_B_MOLS, r, :],
                    start=(r == 0),
                    stop=(r == K - 1),
                )
            o_b = outp.tile([128, HIDDEN], f32, tag="out")
            nc.scalar.activation(
                o_b[:TAIL_B_MOLS, :], ps_b[:TAIL_B_MOLS, :], copy, scale=inv_k
            )
            nc.scalar.dma_start(
                out=y[m0 : m0 + TAIL_B_MOLS, :], in_=o_b[:TAIL_B_MOLS, :]
            )

    nc.finalize()
    return nc


def _get_program():
    if "nc" not in _CACHE:
        _CACHE["nc"] = _build_program()
    return _CACHE["nc"]


def _uniform_pattern(segment_ids: np.ndarray, n_mols: int) -> bool:
    if segment_ids.shape != (TOTAL_ATOMS,) or n_mols != N_MOLS:
        return False
    expect = np.repeat(np.arange(N_MOLS, dtype=segment_ids.dtype), K)
    return bool(np.array_equal(segment_ids, expect))


def _numpy_fallback(atom_hiddens, segment_ids, n_mols):
    """Correct-but-slow path for non-uniform segment layouts (sorted ids)."""
    ah = np.asarray(atom_hiddens, dtype=np.float32)
    sid = np.asarray(segment_ids).astype(np.int64)
    counts = np.bincount(sid, minlength=n_mols).astype(np.float32)
    boundaries = np.searchsorted(sid, np.arange(n_mols))
    sums = np.add.reduceat(ah, boundaries, axis=0)
    empty = counts == 0
    if empty.any():
        sums[empty] = 0.0
    return sums / np.maximum(counts, 1.0)[:, None]


def kernel(**inputs) -> np.ndarray:
    atom_hiddens = np.asarray(inputs["atom_hiddens"], dtype=np.float32)
    segment_ids = np.asarray(inputs["segment_ids"])
    n_mols = int(np.asarray(inputs["n_mols"]))

    if not _uniform_pattern(segment_ids, n_mols) or atom_hiddens.shape != (
        TOTAL_ATOMS,
        HIDDEN,
    ):
        return _numpy_fallback(atom_hiddens, segment_ids, n_mols)

    from concourse.bass_utils import run_bass_kernel_spmd

    nc = _get_program()
    ident = np.eye(128, dtype=np.float32)
    in_maps = [
        {
            "x": atom_hiddens[c * ATOMS_PER_CORE : (c + 1) * ATOMS_PER_CORE],
            "ident": ident,
        }
        for c in range(N_CORES)
    ]
    res = run_bass_kernel_spmd(nc, in_maps, core_ids=list(range(N_CORES)))
    return np.concatenate([r["y"] for r in res.results], axis=0)


if __name__ == "__main__":
    rng = np.random.default_rng(0)
    ah = rng.standard_normal((TOTAL_ATOMS, HIDDEN), dtype=np.float32)
    sid = np.repeat(np.arange(N_MOLS, dtype=np.int32), K)
    out = kernel(atom_hiddens=ah, segment_ids=sid, n_mols=N_MOLS)
    ref = ah.reshape(N_MOLS, K, HIDDEN).mean(axis=1)
    err = np.abs(out - ref).max() / max(np.abs(ref).max(), 1e-9)
    print("rel err:", err)



# revision 12
# speedup vs baseline: 1.1270x; 1.0013x over previous
"""Segment-mean (MeanAggregator) Trainium2 kernel.

Problem: atom_hiddens [2_000_000, 128] f32, segment_ids = repeat(arange(100_000), 20)
(uniform 20 atoms per molecule), output = per-molecule mean [100_000, 128] f32.

Strategy (8 NeuronCores, data-parallel over molecules):
  - Each core handles 12_500 molecules = 250_000 contiguous atom rows (128 MB).
  - Memory-bound: the only thing that matters is keeping the 16 SDMA engines
    (~26 GB/s each while streaming) 100% busy on the input queue.
  - Layout [p, j, r, h]: partition p holds molecules p*G+j of the super-tile as
    one fully-contiguous 20 KB run -> large descriptors, near-peak HBM rate.
    Output [p, j, h] gives 1 KB contiguous runs per partition.
  - Small super-tiles (G=2 groups, 2.62 MB) with 4-deep input pools: DMA for
    ST s+8 depends on compute of ST s (4 rounds back), so the input queue
    never waits on compute (with 2-deep pools the queue idled 13-15 us per
    round waiting for buffers to free).
  - Reduction alternates whole STs between PE and DVE so both streams stay
    well under the DMA pace (~12.8 us per 2-ST round):
      * PE: 20 accumulating float32r matmuls (1 cyc/row at FD=256, vs 4 for
        fp32) with an identity stationary operand; ScalarE evicts * 1/20.
      * DVE: tensor_reduce over [p, h, r] (axis=X), then * 1/20.
    float32r loses a few mantissa bits in the 1.0*x products; tolerance is
    2e-2 so this is far inside budget.
  - Tail (212 mols = 128 + 84) is processed FIRST so the kernel's critical
    path ends with the last main super-tile, not a serial tail.
"""

import numpy as np

N_CORES = 8
TOTAL_ATOMS = 2_000_000
HIDDEN = 128
N_MOLS = 100_000
K = 20  # atoms per molecule
MOLS_PER_CORE = N_MOLS // N_CORES  # 12_500
ATOMS_PER_CORE = TOTAL_ATOMS // N_CORES  # 250_000

G = 2  # groups per super-tile
MOLS_PER_GROUP = 128
ATOMS_PER_GROUP = MOLS_PER_GROUP * K  # 2560
MOLS_PER_ST = G * MOLS_PER_GROUP  # 256
ATOMS_PER_ST = G * ATOMS_PER_GROUP  # 5120
N_ST = MOLS_PER_CORE // MOLS_PER_ST  # 48 full super-tiles
TAIL_MOLS = MOLS_PER_CORE - N_ST * MOLS_PER_ST  # 212
TAIL_A_MOLS = 128
TAIL_B_MOLS = TAIL_MOLS - TAIL_A_MOLS  # 84

_CACHE = {}


def _build_program():
    import concourse.bacc as bacc
    import concourse.tile as tile
    from concourse import mybir

    nc = bacc.Bacc("TRN2", target_bir_lowering=False, debug=False)

    f32 = mybir.dt.float32
    f32r = mybir.dt.float32r

    x = nc.dram_tensor("x", [ATOMS_PER_CORE, HIDDEN], f32r, kind="ExternalInput")
    ident = nc.dram_tensor("ident", [128, 128], f32r, kind="ExternalInput")
    y = nc.dram_tensor("y", [MOLS_PER_CORE, HIDDEN], f32, kind="ExternalOutput")

    inv_k = 1.0 / K
    copy = mybir.ActivationFunctionType.Copy
    AX = mybir.AxisListType.X

    # Tail region lives at the END of this core's atom/mol range but is
    # processed first.
    tail_a0 = N_ST * ATOMS_PER_ST
    tail_m0 = N_ST * MOLS_PER_ST

    with tile.TileContext(nc) as tc:
        with (
            tc.tile_pool(name="constp", bufs=1) as constp,
            tc.tile_pool(name="inp_pe", bufs=4) as inp_pe,
            tc.tile_pool(name="inp_dve", bufs=4) as inp_dve,
            tc.tile_pool(name="outp", bufs=4) as outp,
            tc.tile_pool(name="tailp", bufs=1) as tailp,
            tc.tile_pool(name="psump", bufs=4, space="PSUM") as psump,
            tc.tile_pool(name="tailps", bufs=1, space="PSUM") as tailps,
        ):
            ident_sb = constp.tile([128, 128], f32r)
            nc.scalar.dma_start(out=ident_sb, in_=ident[:, :])
            ident_r = ident_sb

            # ---- tail A: 128 mols ----
            in_a = tailp.tile([128, K, HIDDEN], f32r)
            nc.sync.dma_start(
                out=in_a,
                in_=x[tail_a0 : tail_a0 + ATOMS_PER_GROUP, :].rearrange(
                    "(p r) h -> p r h", p=128, r=K
                ),
            )
            ps_a = tailps.tile([128, HIDDEN], f32)
            for r in range(K):
                nc.tensor.matmul(
                    ps_a,
                    lhsT=ident_r,
                    rhs=in_a[:, r, :],
                    start=(r == 0),
                    stop=(r == K - 1),
                )
            o_a = outp.tile([128, HIDDEN], f32, tag="out")
            nc.scalar.activation(o_a, ps_a, copy, scale=inv_k)
            nc.scalar.dma_start(out=y[tail_m0 : tail_m0 + TAIL_A_MOLS, :], in_=o_a)

            # ---- tail B: 84 mols ----
            b_a0 = tail_a0 + ATOMS_PER_GROUP
            b_m0 = tail_m0 + TAIL_A_MOLS
            in_b = tailp.tile([128, K, HIDDEN], f32r)
            nc.sync.dma_start(
                out=in_b[:TAIL_B_MOLS],
                in_=x[b_a0 : b_a0 + TAIL_B_MOLS * K, :].rearrange(
                    "(p r) h -> p r h", p=TAIL_B_MOLS, r=K
                ),
            )
            ps_b = tailps.tile([128, HIDDEN], f32)
            for r in range(K):
                nc.tensor.matmul(
                    ps_b[:TAIL_B_MOLS, :],
                    lhsT=ident_r[:TAIL_B_MOLS, :TAIL_B_MOLS],
                    rhs=in_b[:TAIL_B_MOLS, r, :],
                    start=(r == 0),
                    stop=(r == K - 1),
                )
            o_b = outp.tile([128, HIDDEN], f32, tag="out")
            nc.scalar.activation(
                o_b[:TAIL_B_MOLS, :], ps_b[:TAIL_B_MOLS, :], copy, scale=inv_k
            )
            nc.scalar.dma_start(
                out=y[b_m0 : b_m0 + TAIL_B_MOLS, :], in_=o_b[:TAIL_B_MOLS, :]
            )

            # ---- main super-tiles ----
            for s in range(N_ST):
                a0 = s * ATOMS_PER_ST
                on_pe = s % 2 == 0
                pool = inp_pe if on_pe else inp_dve
                in_t = pool.tile([128, G, K, HIDDEN], f32r)
                nc.sync.dma_start(
                    out=in_t,
                    in_=x[a0 : a0 + ATOMS_PER_ST, :].rearrange(
                        "(p j r) h -> p j r h", p=128, j=G, r=K
                    ),
                )

                o_t = outp.tile([128, G, HIDDEN], f32, tag="out")

                if on_pe:
                    ps = psump.tile([128, G * HIDDEN], f32)
                    for r in range(K):
                        nc.tensor.matmul(
                            ps,
                            lhsT=ident_r,
                            rhs=in_t[:, :, r, :],
                            start=(r == 0),
                            stop=(r == K - 1),
                        )
                    nc.scalar.activation(o_t, ps, copy, scale=inv_k)
                else:
                    for j in range(G):
                        nc.vector.reduce_sum(
                            out=o_t[:, j, :],
                            in_=in_t[:, j, :, :].bitcast(f32).rearrange("p r h -> p h r"),
                            axis=AX,
                        )
                    nc.vector.tensor_scalar_mul(o_t, o_t, inv_k)

                dst = y[s * MOLS_PER_ST : (s + 1) * MOLS_PER_ST, :].rearrange(
                    "(p j) h -> p j h", p=128, j=G
                )
                nc.scalar.dma_start(out=dst, in_=o_t)

    nc.finalize()
    return nc


def _get_program():
    if "nc" not in _CACHE:
        _CACHE["nc"] = _build_program()
    return _CACHE["nc"]


def _uniform_pattern(segment_ids: np.ndarray, n_mols: int) -> bool:
    if segment_ids.shape != (TOTAL_ATOMS,) or n_mols != N_MOLS:
        return False
    expect = np.repeat(np.arange(N_MOLS, dtype=segment_ids.dtype), K)
    return bool(np.array_equal(segment_ids, expect))


def _numpy_fallback(atom_hiddens, segment_ids, n_mols):
    """Correct-but-slow path for non-uniform segment layouts (sorted ids)."""
    ah = np.asarray(atom_hiddens, dtype=np.float32)
    sid = np.asarray(segment_ids).astype(np.int64)
    counts = np.bincount(sid, minlength=n_mols).astype(np.float32)
    boundaries = np.searchsorted(sid, np.arange(n_mols))
    sums = np.add.reduceat(ah, boundaries, axis=0)
    empty = counts == 0
    if empty.any():
        sums[empty] = 0.0
    return sums / np.maximum(counts, 1.0)[:, None]


def kernel(**inputs) -> np.ndarray:
    atom_hiddens = np.asarray(inputs["atom_hiddens"], dtype=np.float32)
    segment_ids = np.asarray(inputs["segment_ids"])
    n_mols = int(np.asarray(inputs["n_mols"]))

    if not _uniform_pattern(segment_ids, n_mols) or atom_hiddens.shape != (
        TOTAL_ATOMS,
        HIDDEN,
    ):
        return _numpy_fallback(atom_hiddens, segment_ids, n_mols)

    from concourse.bass_utils import run_bass_kernel_spmd

    nc = _get_program()
    ident = np.eye(128, dtype=np.float32)
    in_maps = [
        {
            "x": atom_hiddens[c * ATOMS_PER_CORE : (c + 1) * ATOMS_PER_CORE],
            "ident": ident,
        }
        for c in range(N_CORES)
    ]
    res = run_bass_kernel_spmd(nc, in_maps, core_ids=list(range(N_CORES)))
    return np.concatenate([r["y"] for r in res.results], axis=0)


if __name__ == "__main__":
    rng = np.random.default_rng(0)
    ah = rng.standard_normal((TOTAL_ATOMS, HIDDEN), dtype=np.float32)
    sid = np.repeat(np.arange(N_MOLS, dtype=np.int32), K)
    out = kernel(atom_hiddens=ah, segment_ids=sid, n_mols=N_MOLS)
    ref = ah.reshape(N_MOLS, K, HIDDEN).mean(axis=1)
    err = np.abs(out - ref).max() / max(np.abs(ref).max(), 1e-9)
    print("rel err:", err)
